# revision 16
# baseline (speedup 1.0000x reference)
"""Trainium2 Bass kernel for windowed local self-attention MLP.

Reference computation (per batch b, S=2048 tokens, D=H=256, A=16, W=33):
    h   = relu(x @ W1 + b1)
    Q   = h @ Wq ; K = h @ Wk ; V = h @ Wv  (windowed K/V are shifted views)
    logit[s,m] = Q[s].K[s+A-m]/sqrt(H)  (zero outside [0,S), m=0..32)
    attn = softmax(logit) ; att[s] = sum_m attn[s,m] V[s+A-m]
    out = relu(att @ Wh + bh) @ Wo + bo

Sharding: data-parallel over batch, one batch element per NeuronCore (B=8,
8 cores), weights replicated, no collectives.

Algebraic folds (host, float64): M = Wq @ Wk^T so K is h itself, and
Wv @ Wh so the attention-apply emits pre-relu hid directly. All activations
and weights are fp16 on-chip (PSUM accumulation stays fp32); only PSUM
drains and biases are fp32.

Layout: feature-on-partition for dense matmuls; h is stored once as a
zero-padded fp16 [256=2x128, 2176] tile (col = token + A) and serves as
both matmul input and the attention K. Band attention runs per 128-token
chunk over a 160-token window; the additive band mask is matmul-preloaded
into PSUM, fp16 QK logits accumulate on top, one exp per chunk-pair feeds
a DVE row-sum + divide normalization, PE transposes of the fp16 weights
feed V^T matmuls whose PSUM is relu'd into hid.

Schedule: phase-major (all p1 stripes, all qt stripes, all V tiles, then a
software-pipelined attention+output wave). A stream of tiny warm-up
matmuls keeps the PE busy from t~=0.3us so the tensor clock is fully
ramped when real work arrives, and every PSUM drain is placed on
ACT/DVE so the PE never waits on a drain engine.
"""
import sys

if "/opt/trn_rl_repo" not in sys.path:
    sys.path.insert(0, "/opt/trn_rl_repo")

import numpy as np

import concourse.mybir as mybir
import concourse.tile as tile
from concourse import bacc
from concourse.bass_utils import run_bass_kernel_spmd

P = 128
S = 2048  # tokens per core
D = 256  # model dim
A = 16  # half window
NC = 16  # token chunks per core
NCORES = 8

PADW = P * (NC + 1)  # 2176: padded token axis, col = token + A
WINW = P + 2 * A  # 160: per-chunk attention window
F32 = mybir.dt.float32
FP16 = mybir.dt.float16

NDUMMY = 72  # PE warm-up matmuls (keep clock ramped through DMA wait)

_CACHED_NC = None
_LAST_RESULTS = None


def _build_nc():
    nc = bacc.Bacc(
        "TRN2",
        target_bir_lowering=False,
        debug=False,
        enable_asserts=False,
        num_devices=NCORES,
    )
    xt = nc.dram_tensor("xt", [D, S], FP16, kind="ExternalInput").ap()
    w1 = nc.dram_tensor("w1", [D, D], FP16, kind="ExternalInput").ap()
    wq = nc.dram_tensor("wq", [D, D], FP16, kind="ExternalInput").ap()
    wv = nc.dram_tensor("wv", [D, D], FP16, kind="ExternalInput").ap()
    bias4 = nc.dram_tensor("bias4", [P, 4], F32, kind="ExternalInput").ap()
    # hpk: idh [P,128] | mkb [P,320] | wo [P,4]
    hpk = nc.dram_tensor("hpk", [P, 452], FP16, kind="ExternalInput").ap()
    out_t = nc.dram_tensor("out_t", [2, S], F32, kind="ExternalOutput").ap()

    with tile.TileContext(nc) as tc:
        with (
            tc.tile_pool(name="persist", bufs=1) as persist,
            tc.tile_pool(name="work", bufs=8) as work,
            tc.tile_pool(name="psum", bufs=8, space="PSUM") as psum,
        ):
            # ---------------- persistent tiles ----------------
            w1_sb = persist.tile([P, 2, D], FP16)
            wq_sb = persist.tile([P, 2, D], FP16)
            wv_sb = persist.tile([P, 2, D], FP16)
            bias_sb = persist.tile([P, 4], F32)  # b1 lo/hi | bh lo/hi
            hpk_sb = persist.tile([P, 452], FP16)
            id_h = hpk_sb[:, 0:P]
            mk_h = hpk_sb[:, P:P + 2 * WINW]
            wo_sb = hpk_sb[:, P + 2 * WINW:P + 2 * WINW + 4]

            xt_sb = persist.tile([P, 2, S], FP16)
            ht = persist.tile([P, 2, PADW], FP16)  # col = token + A
            qt = persist.tile([P, 2, S], FP16)
            vs = persist.tile([P, NC + 1, D], FP16)  # tile t row p = tok t*128+p-A
            hid = persist.tile([P, 2, S], FP16)
            ot_sb = persist.tile([2, S], F32)
            dmy = persist.tile([P, 64], FP16)

            # ---- startup: memsets, DMAs, PE warm-up ----
            # Transfer order on the single DMA pipe is descriptor-ready
            # order, so the two tensors gating p1(0) go first.
            nc.gpsimd.memset(dmy[:], 0.0)
            xtr = xt.rearrange("(ko p) s -> p ko s", p=P)

            def rearr(w):
                return w.rearrange("(k p) h -> p k h", p=P)

            nc.sync.dma_start(w1_sb[:], rearr(w1))
            nc.scalar.dma_start(xt_sb[:, :, 0:512], xtr[:, :, 0:512])
            nc.scalar.dma_start(bias_sb[:], bias4)
            for t in range(1, 4):
                nc.sync.dma_start(
                    xt_sb[:, :, t * 512:(t + 1) * 512],
                    xtr[:, :, t * 512:(t + 1) * 512],
                )
            nc.scalar.dma_start(wq_sb[:], rearr(wq))
            nc.scalar.dma_start(wv_sb[:], rearr(wv))
            for ko in range(2):
                nc.gpsimd.memset(ht[:, ko, 0:A], 0.0)
                nc.gpsimd.memset(ht[:, ko, S + A:PADW], 0.0)
            nc.gpsimd.dma_start(hpk_sb[:], hpk)

            for _ in range(NDUMMY):
                psd = psum.tile([64, 64], F32, tag="bank", name="warm")
                nc.tensor.matmul(
                    psd[:], dmy[0:64, 0:64], dmy[0:64, 0:64],
                    start=True, stop=True,
                )

            # ---------------- dense phase bodies ----------------
            def p1_stripe(t):  # ht = relu(W1^T @ xt + b1), 512 tokens
                for hm in range(2):
                    ps = psum.tile([P, 512], F32, tag="bank")
                    for k in range(2):
                        nc.tensor.matmul(
                            ps[:], w1_sb[:, k, hm * P:(hm + 1) * P],
                            xt_sb[:, k, t * 512:(t + 1) * 512],
                            start=(k == 0), stop=(k == 1),
                        )
                    dst = ht[:, hm, A + t * 512:A + (t + 1) * 512]
                    if hm == 0:
                        nc.scalar.activation(
                            dst, ps[:], mybir.ActivationFunctionType.Relu,
                            bias=bias_sb[:, 0:1],
                        )
                    else:
                        nc.vector.tensor_scalar(
                            dst, ps[:], bias_sb[:, 1:2], 0.0,
                            mybir.AluOpType.add, mybir.AluOpType.max,
                        )

            def p23_stripe(t):  # qt = M^T ht (M = Wq Wk^T, host-folded)
                for hm in range(2):
                    psq = psum.tile([P, 512], F32, tag="bank")
                    for k in range(2):
                        nc.tensor.matmul(
                            psq[:], wq_sb[:, k, hm * P:(hm + 1) * P],
                            ht[:, k, A + t * 512:A + (t + 1) * 512],
                            start=(k == 0), stop=(k == 1),
                        )
                    dst = qt[:, hm, t * 512:(t + 1) * 512]
                    if hm == 0:
                        nc.scalar.copy(dst, psq[:])
                    else:
                        nc.vector.tensor_copy(dst, psq[:])

            p4_alt = [0]

            def p4_group(v0, n):  # shifted V tiles (natural layout, fp16)
                psv = psum.tile([P, 2, D], F32, tag="bank")
                for i in range(n):
                    for k in range(2):
                        nc.tensor.matmul(
                            psv[:, i, :],
                            ht[:, k, (v0 + i) * P:(v0 + i + 1) * P],
                            wv_sb[:, k, :],
                            start=(k == 0), stop=(k == 1),
                        )
                if p4_alt[0] % 2 == 0:
                    nc.scalar.copy(vs[:, v0:v0 + n, :], psv[:, 0:n, :])
                else:
                    nc.vector.tensor_copy(vs[:, v0:v0 + n, :], psv[:, 0:n, :])
                p4_alt[0] += 1

            # ---------------- attention stage bodies ----------------
            pair_state = {}

            def p5_logits(cp):  # PE: mask init + QK logits for both chunks
                psl = psum.tile([P, 2 * WINW], F32, tag="bank", name="logit")
                nc.tensor.matmul(psl[:], id_h, mk_h, start=True, stop=False)
                for ci in range(2):
                    c = 2 * cp + ci
                    for k in range(2):
                        nc.tensor.matmul(
                            psl[:, ci * WINW:(ci + 1) * WINW],
                            qt[:, k, c * P:(c + 1) * P],
                            ht[:, k, c * P:c * P + WINW],
                            start=False, stop=(ci == 1 and k == 1),
                        )
                pair_state[cp] = psl

            def p5_exp(cp):  # ACT: one fused exp over both chunks
                psl = pair_state.pop(cp)
                e = work.tile([P, 2, WINW], FP16, tag="e")
                nc.scalar.activation(
                    e[:], psl[:], mybir.ActivationFunctionType.Exp,
                    scale=0.0625,
                )
                pair_state[("e", cp)] = e

            def p5_norm(cp):  # DVE rowsum + recip, Pool normalize
                e = pair_state.pop(("e", cp))
                den = work.tile([P, 2], F32, tag="den")
                nc.vector.tensor_reduce(
                    den[:], e[:], mybir.AxisListType.X, mybir.AluOpType.add
                )
                rec = work.tile([P, 2], F32, tag="rec")
                nc.vector.reciprocal(rec[:], den[:])
                enb = work.tile([P, 2, WINW], FP16, tag="enb")
                # Pool normalize while the wave is full; DVE (shorter hop)
                # for the drain-phase pairs where DVE has slack
                eng = nc.vector if cp >= 5 else nc.gpsimd
                for ci in range(2):
                    eng.tensor_scalar_mul(
                        enb[:, ci], e[:, ci], rec[:, ci:ci + 1]
                    )
                pair_state[("enb", cp)] = enb

            def p5_transpose(cp):  # PE transposes of normalized weights
                enb = pair_state.pop(("enb", cp))
                pse = psum.tile([P, 4, P], FP16, tag="bank", name="etr")
                for ci in range(2):
                    nc.tensor.transpose(
                        pse[:, 2 * ci, :], enb[:, ci, 0:P], id_h
                    )
                    nc.tensor.transpose(
                        pse[0:2 * A, 2 * ci + 1, :], enb[:, ci, P:WINW], id_h
                    )
                pair_state[("pse", cp)] = pse

            def p5_etcopy(cp):  # DVE: drain transposed weights to SBUF
                pse = pair_state.pop(("pse", cp))
                et = work.tile([P, 4, P], FP16, tag="et")
                nc.vector.tensor_copy(et[:], pse[:])
                pair_state[("et", cp)] = et

            def p5_apply(cp):  # PE V-apply + hid drains (ACT/DVE)
                et = pair_state.pop(("et", cp))
                psa = psum.tile([P, 2, 2 * P], F32, tag="bank", name="attp")
                for ci in range(2):
                    c = 2 * cp + ci
                    for fm in range(2):
                        nc.tensor.matmul(
                            psa[:, fm, ci * P:(ci + 1) * P],
                            vs[:, c, fm * P:(fm + 1) * P],
                            et[:, 2 * ci, :],
                            start=True, stop=False,
                        )
                        nc.tensor.matmul(
                            psa[:, fm, ci * P:(ci + 1) * P],
                            vs[0:2 * A, c + 1, fm * P:(fm + 1) * P],
                            et[0:2 * A, 2 * ci + 1, :],
                            start=False, stop=True,
                        )
                nc.vector.tensor_scalar(
                    hid[:, 0, cp * 2 * P:(cp + 1) * 2 * P], psa[:, 0, :],
                    bias_sb[:, 2:3], 0.0,
                    mybir.AluOpType.add, mybir.AluOpType.max,
                )
                nc.scalar.activation(
                    hid[:, 1, cp * 2 * P:(cp + 1) * 2 * P], psa[:, 1, :],
                    mybir.ActivationFunctionType.Relu, bias=bias_sb[:, 3:4],
                )

            p7_alt = [0]

            def p7_piece(u, hh=0, w=512):  # out^T = Wo^T @ hid, stream out
                lo = u * 512 + hh * w
                pso = psum.tile([2, 512], F32, tag="bank", name="outp")
                for k in range(2):
                    nc.tensor.matmul(
                        pso[:, 0:w], wo_sb[:, k * 2:k * 2 + 2],
                        hid[:, k, lo:lo + w],
                        start=(k == 0), stop=(k == 1),
                    )
                if p7_alt[0] % 2 == 0:
                    nc.scalar.copy(ot_sb[:, lo:lo + w], pso[:, 0:w])
                else:
                    nc.vector.tensor_copy(ot_sb[:, lo:lo + w], pso[:, 0:w])
                p7_alt[0] += 1
                nc.sync.dma_start(out_t[:, lo:lo + w], ot_sb[:, lo:lo + w])

            # ---------------- unified wave emission ----------------
            # One continuous slot table: the dense stripes, V tiles,
            # attention stages and output pieces interleave so no engine
            # sees a phase boundary. Stage lags (T = L+2, A = T+1) cover
            # the cross-engine softmax round trip.
            def stage(kind, i):
                if kind == "p1":
                    p1_stripe(i)
                elif kind == "qt":
                    p23_stripe(i)
                elif kind == "v":
                    p4_group(2 * i, 2) if i < 8 else p4_group(NC, 1)
                elif kind == "L":
                    p5_logits(i)
                    p5_exp(i)
                elif kind == "N":
                    p5_norm(i)
                elif kind == "T":
                    p5_transpose(i)
                    p5_etcopy(i)
                elif kind == "A":
                    p5_apply(i)

            slots = [
                [("p1", 0)],
                [("p1", 1)],
                [("qt", 0), ("v", 0)],
                [("p1", 2), ("L", 0), ("v", 1)],
                [("qt", 1), ("N", 0), ("v", 2)],
                [("p1", 3), ("L", 1), ("T", 0), ("v", 3)],
                [("qt", 2), ("L", 2), ("A", 0), ("N", 1)],
                [("qt", 3), ("L", 3), ("N", 2), ("T", 1), ("v", 4)],
                [("L", 4), ("A", 1), ("N", 3), ("T", 2), ("v", 5)],
                [("L", 5), ("A", 2), ("N", 4), ("T", 3), ("v", 6)],
                [("L", 6), ("A", 3), ("N", 5), ("T", 4), ("v", 7)],
                [("L", 7), ("A", 4), ("N", 6), ("T", 5), ("v", 8)],
                [("A", 5), ("N", 7), ("T", 6)],
                [("A", 6), ("T", 7)],
                [("A", 7)],
            ]
            for m, slot in enumerate(slots):
                for kind, i in slot:
                    stage(kind, i)
                # out pieces ride the drain-phase slots, padding the PE
                # while the last pairs' softmax chains complete
                if m == 12:
                    p7_piece(0)
                if m == 13:
                    p7_piece(1)
                if m == 14:
                    p7_piece(2)
            p7_piece(3, 0, 384)
            p7_piece(3, 1, 128)

    nc.compile()
    return nc


def _get_nc():
    global _CACHED_NC
    if _CACHED_NC is None:
        _CACHED_NC = _build_nc()
    return _CACHED_NC


def _band_mask():
    j = np.arange(WINW)[None, :]
    p = np.arange(P)[:, None]
    m = np.where((j >= p) & (j <= p + 2 * A), 0.0, -60000.0).astype(np.float16)
    return np.tile(m, (1, 2))


def kernel(x, W1, b1, Wq, Wk, Wv, Wh, bh, Wo, bo, **_unused):
    x = np.asarray(x, dtype=np.float32)
    W1 = np.asarray(W1, dtype=np.float32)
    Wq = np.asarray(Wq, dtype=np.float32)
    Wk = np.asarray(Wk, dtype=np.float32)
    Wv = np.asarray(Wv, dtype=np.float32)
    Wh = np.asarray(Wh, dtype=np.float32)
    Wo = np.asarray(Wo, dtype=np.float32)
    b1f = np.asarray(b1, dtype=np.float32).reshape(D)
    bhf = np.asarray(bh, dtype=np.float32).reshape(D)
    bof = np.asarray(bo, dtype=np.float32).reshape(2)

    wqm = (Wq.astype(np.float64) @ Wk.astype(np.float64).T).astype(np.float16)
    wvh = (Wv.astype(np.float64) @ Wh.astype(np.float64)).astype(np.float16)

    bias4 = np.stack(
        [b1f[:P], b1f[P:], bhf[:P], bhf[P:]], axis=1
    ).astype(np.float32)
    wo_re = np.ascontiguousarray(
        Wo.reshape(2, P, 2).transpose(1, 0, 2).reshape(P, 4)
    ).astype(np.float16)
    hpk = np.concatenate(
        [np.eye(P, dtype=np.float16), _band_mask(), wo_re], axis=1
    ).astype(np.float16)

    nc = _get_nc()
    in_maps = []
    for b in range(NCORES):
        in_maps.append({
            "xt": np.ascontiguousarray(x[b].T).astype(np.float16),
            "w1": W1.astype(np.float16),
            "wq": wqm, "wv": wvh,
            "bias4": bias4, "hpk": hpk,
        })
    # one retry: the shared device occasionally throws a transient
    # NRT_EXEC_UNIT_UNRECOVERABLE; re-running recovers it
    try:
        res = run_bass_kernel_spmd(nc, in_maps, core_ids=list(range(NCORES)))
    except Exception:
        res = run_bass_kernel_spmd(nc, in_maps, core_ids=list(range(NCORES)))
    global _LAST_RESULTS
    _LAST_RESULTS = res
    out = np.stack(
        [res.results[b]["out_t"].T + bof[None, :] for b in range(NCORES)], axis=0
    )
    return out.astype(np.float32)


if __name__ == "__main__":
    rng = np.random.default_rng(0)
    ins = {
        "x": rng.standard_normal((8, S, D), dtype=np.float32),
        "W1": (rng.standard_normal((D, D), dtype=np.float32) / 16),
        "b1": np.zeros((1, 1, D), np.float32),
        "Wq": (rng.standard_normal((D, D), dtype=np.float32) / 16),
        "Wk": (rng.standard_normal((D, D), dtype=np.float32) / 16),
        "Wv": (rng.standard_normal((D, D), dtype=np.float32) / 16),
        "Wh": (rng.standard_normal((D, D), dtype=np.float32) / 16),
        "bh": np.zeros((1, 1, D), np.float32),
        "Wo": (rng.standard_normal((D, 2), dtype=np.float32) / 16),
        "bo": np.zeros((1, 1, 2), np.float32),
    }
    y = kernel(**ins)
    print("kernel output", y.shape, y.dtype, float(np.abs(y).max()))
